# revision 17
# baseline (speedup 1.0000x reference)
"""BlurredPhonemeEmbedding TRN2 kernel v2 — windowed step-matmul expansion.

Full inputs: ids (32, 8192) int64, table (2820, 64) f32.
Output: (32, 8192, 64) f32 = (1-w)*tbl[ids] + w*tbl[neighbor].

Data-parallel over batch: 8 cores x 4 rows; table replicated.

Device algorithm per core (R=4 rows, T=8192):
 - scan layout [128, 256]: partition p = r*32 + c, free j; t_row = c*256 + j.
 - segment quantities (start/end/dur/dur_prev/dur_next) via masked
   fill-forward scans with cross-chunk carries (as v1), minus the id scans.
 - blend weight w and neighbor-representative position g per position.
 - HOST preps per-(row,group) segment windows (pure index data):
   group = 512 positions, window = 128 segment slots [s0-1, s0+126];
   widx: wrapped int16 ids for ONE dma_gather of 64*128 window rows;
   bneg[j, ga] = 32*(0.5 - (sstart[s] - 512*g)) for ACT sigmoid bias.
 - emb[t] = sum_j step(q >= b_j) * Delta_j  (telescoping over window rows),
   nemb[t] same with g_rel instead of q. Steps built on ACT via saturated
   sigmoid (scale 32 -> exact 0/1 in fp16); expansion via PE fp16 matmuls
   accumulating f32 in PSUM. Blend on DVE in f32; per-group 128KB stores.
"""
import numpy as np

import concourse.bass as bass
import concourse.tile as tile
from concourse import bacc, mybir
from concourse.bass_utils import run_bass_kernel_spmd
from concourse.masks import make_identity

F32 = mybir.dt.float32
F16 = mybir.dt.float16
I32 = mybir.dt.int32
I16 = mybir.dt.int16
OP = mybir.AluOpType

B, T, V, D = 32, 8192, 2820, 64
NCORES = 8
R = B // NCORES            # rows per core = 4
P = 128                    # partitions
CPR = P // R               # chunks per row = 32
CL = T // CPR              # chunk length = 256
GP = 512                   # positions per group
NGR = T // GP              # groups per row = 16
NGA = R * NGR              # groups per core = 64
WIN = 128                  # window slots per group
MAGIC = float(2 ** 23)
BIGNEG = -2048.0
SC = 32.0                  # sigmoid step scale


def build_nc(dbg=False):
    nc = bacc.Bacc("TRN2", target_bir_lowering=False, debug=False,
                   dynamic_dma_scratch_size=16384)
    ids_d = nc.dram_tensor("ids", [R, T], I32, kind="ExternalInput")
    widx_d = nc.dram_tensor("widx", [P, NGA * WIN // 16], I16, kind="ExternalInput")
    bneg_d = nc.dram_tensor("bneg", [P, NGA], F32, kind="ExternalInput")
    tbl_d = nc.dram_tensor("table", [V, D], F32, kind="ExternalInput")
    out_d = nc.dram_tensor("out", [R, T, D], F32, kind="ExternalOutput")

    with tile.TileContext(nc) as tc:
        with tc.tile_pool(name="main", bufs=1) as mp, \
             tc.tile_pool(name="sg", bufs=6) as sgp, \
             tc.tile_pool(name="ob", bufs=6) as obp, \
             tc.tile_pool(name="ps", bufs=3, space="PSUM") as pp, \
             tc.tile_pool(name="psg", bufs=2, space="PSUM") as pgp:

            def t256(name, dt=F32):
                return mp.tile([P, CL], dt, name=name, tag=name)

            # ---------- loads ----------
            ids_i = t256("ids_i", I32)
            nc.sync.dma_start(ids_i[:], ids_d[:].rearrange("r (c j) -> (r c) j", j=CL))
            widx = mp.tile([P, NGA * WIN // 16], I16, name="widx_t", tag="widx_t")
            nc.sync.dma_start(widx[:], widx_d[:])
            bneg = mp.tile([P, NGA], F32, name="bneg_t", tag="bneg_t")
            nc.sync.dma_start(bneg[:], bneg_d[:])

            # static iotas (gpsimd first, before the big gather)
            pos_i = t256("pos_i", I32)
            nc.gpsimd.iota(pos_i[:], pattern=[[1, CL]], base=0, channel_multiplier=CL)
            iq_i = mp.tile([P, GP], I32, name="iq_i", tag="iq_i")
            nc.gpsimd.iota(iq_i[:], pattern=[[1, GP]], base=0, channel_multiplier=0)
            iq = mp.tile([P, GP], F32, name="iq", tag="iq")
            nc.vector.tensor_copy(out=iq[:], in_=iq_i[:])
            ip_i = mp.tile([P, 1], I32, name="ip_i", tag="ip_i")
            nc.gpsimd.iota(ip_i[:], pattern=[[0, 1]], base=0, channel_multiplier=1)
            ip = mp.tile([P, 1], F32, name="ip", tag="ip")
            nc.vector.tensor_copy(out=ip[:], in_=ip_i[:])

            # Window-row gather happens in 4 chunks of 2048 rows, issued AFTER
            # the scan pipeline (see below): the gpsimd engine is in-order, so
            # a tiny gpsimd op depending on the last scan output fences the
            # gathers behind the scans — Q7 SWDGE descriptor generation would
            # otherwise slow the DVE scans 2-4x via SBUF port contention.
            wins = mp.tile([P, NGA * D], F32, name="wins", tag="wins")
            NCH = 4
            GCH = NGA // NCH           # 16 groups per chunk
            # per-chunk delta tiles so early groups only wait on their chunk
            deltas = [mp.tile([P, GCH * D], F16, name=f"delta{ch}",
                              tag=f"delta{ch}") for ch in range(NCH)]

            # ---------- pos / masks / scans (v1 machinery minus id scans) ----
            nc.vector.tensor_scalar(out=pos_i[:], in0=pos_i[:], scalar1=T - 1,
                                    scalar2=None, op0=OP.bitwise_and)
            pos = t256("pos")
            nc.vector.tensor_copy(out=pos[:], in_=pos_i[:])

            ids_f = t256("ids_f")
            nc.vector.tensor_copy(out=ids_f[:], in_=ids_i[:])

            ids_prev = t256("ids_prev")
            nc.vector.memset(ids_prev[:], 0.0)
            nc.vector.tensor_copy(out=ids_prev[:, 1:CL], in_=ids_f[:, 0:CL - 1])
            nc.sync.dma_start(ids_prev[1:P, 0:1], ids_f[0:P - 1, CL - 1:CL])

            ids_next = t256("ids_next")
            nc.vector.memset(ids_next[:], 0.0)
            nc.vector.tensor_copy(out=ids_next[:, 0:CL - 1], in_=ids_f[:, 1:CL])
            nc.sync.dma_start(ids_next[0:P - 1, CL - 1:CL], ids_f[1:P, 0:1])

            m_s = t256("m_s")
            nc.vector.tensor_tensor(out=m_s[:], in0=ids_f[:], in1=ids_prev[:],
                                    op=OP.not_equal)
            edge_s = t256("edge_s")
            nc.vector.tensor_scalar(out=edge_s[:], in0=pos[:], scalar1=0.0,
                                    scalar2=None, op0=OP.is_equal)
            nc.vector.tensor_tensor(out=m_s[:], in0=m_s[:], in1=edge_s[:], op=OP.max)
            m_e = t256("m_e")
            nc.vector.tensor_tensor(out=m_e[:], in0=ids_f[:], in1=ids_next[:],
                                    op=OP.not_equal)
            edge_e = t256("edge_e")
            nc.vector.tensor_scalar(out=edge_e[:], in0=pos[:], scalar1=float(T - 1),
                                    scalar2=None, op0=OP.is_equal)
            nc.vector.tensor_tensor(out=m_e[:], in0=m_e[:], in1=edge_e[:], op=OP.max)

            om_s = t256("om_s")
            nc.vector.tensor_scalar(out=om_s[:], in0=m_s[:], scalar1=-1.0, scalar2=1.0,
                                    op0=OP.mult, op1=OP.add)
            om_e = t256("om_e")
            nc.vector.tensor_scalar(out=om_e[:], in0=m_e[:], scalar1=-1.0, scalar2=1.0,
                                    op0=OP.mult, op1=OP.add)

            def rev(ap):
                return ap[:, CL - 1::-1]

            def ffscan(out_t, d1, initial, backward=False):
                om = om_e if backward else om_s
                if backward:
                    nc.vector.tensor_tensor_scan(
                        out=rev(out_t[:]), data0=rev(om[:]), data1=rev(d1[:]),
                        initial=initial, op0=OP.mult, op1=OP.add)
                else:
                    nc.vector.tensor_tensor_scan(
                        out=out_t[:], data0=om[:], data1=d1[:],
                        initial=initial, op0=OP.mult, op1=OP.add)

            pv_start = t256("pv_start")
            nc.vector.tensor_tensor(out=pv_start[:], in0=pos[:], in1=m_s[:], op=OP.mult)
            pv_end = t256("pv_end")
            nc.vector.scalar_tensor_tensor(out=pv_end[:], in0=pos[:], scalar=1.0,
                                           in1=m_e[:], op0=OP.add, op1=OP.mult)

            s_start = t256("s_start")
            ffscan(s_start, pv_start, 0.0)
            s_end = t256("s_end")
            ffscan(s_end, pv_end, 0.0, backward=True)

            # cross-chunk carries: [128, 4] -> [1, 512] transposed view
            NSC = 4
            coll = mp.tile([P, NSC], F32, name="coll", tag="coll")
            nc.vector.tensor_copy(out=coll[:, 0:1], in_=s_start[:, CL - 1:CL])
            nc.vector.tensor_copy(out=coll[:, 1:2], in_=s_end[:, 0:1])
            nc.vector.tensor_reduce(out=coll[:, 2:3], in_=m_s[:],
                                    axis=mybir.AxisListType.X, op=OP.max)
            nc.vector.tensor_reduce(out=coll[:, 3:4], in_=m_e[:],
                                    axis=mybir.AxisListType.X, op=OP.max)

            crossT = mp.tile([1, NSC * P], F32, name="crossT", tag="crossT")
            nc.sync.dma_start(crossT[0:1, :], coll[:, :])
            crossT_v = crossT[0:1, :].rearrange("a (p k) -> a k p", k=NSC)

            def cslot(k):
                return crossT_v[:, k]

            rr = mp.tile([1, P], F32, name="rr", tag="rr")
            nc.vector.memset(rr[:], 1.0)
            rrb = mp.tile([1, P], F32, name="rrb", tag="rrb")
            nc.vector.memset(rrb[:], 1.0)
            for r in range(R):
                nc.vector.memset(rr[0:1, r * CPR:r * CPR + 1], 0.0)
                nc.vector.memset(rrb[0:1, (r + 1) * CPR - 1:(r + 1) * CPR], 0.0)

            hs_f = mp.tile([1, P], F32, name="hs_f", tag="hs_f")
            nc.vector.memset(hs_f[0:1, 0:1], 0.0)
            nc.vector.tensor_copy(out=hs_f[0:1, 1:P], in_=cslot(2)[0:1, 0:P - 1])
            d0f = mp.tile([1, P], F32, name="d0f", tag="d0f")
            nc.vector.tensor_scalar(out=d0f[:], in0=hs_f[:], scalar1=-1.0, scalar2=1.0,
                                    op0=OP.mult, op1=OP.add)
            nc.vector.tensor_tensor(out=d0f[:], in0=d0f[:], in1=rr[:], op=OP.mult)
            hs_b = mp.tile([1, P], F32, name="hs_b", tag="hs_b")
            nc.vector.memset(hs_b[0:1, P - 1:P], 0.0)
            nc.vector.tensor_copy(out=hs_b[0:1, 0:P - 1], in_=cslot(3)[0:1, 1:P])
            d0b = mp.tile([1, P], F32, name="d0b", tag="d0b")
            nc.vector.tensor_scalar(out=d0b[:], in0=hs_b[:], scalar1=-1.0, scalar2=1.0,
                                    op0=OP.mult, op1=OP.add)
            nc.vector.tensor_tensor(out=d0b[:], in0=d0b[:], in1=rrb[:], op=OP.mult)

            carryT = mp.tile([1, NSC * P], F32, name="carryT", tag="carryT")
            carryT_v = carryT[0:1, :].rearrange("a (p k) -> a k p", k=NSC)

            def cross_fwd(k):
                ss = mp.tile([1, P], F32, name=f"ss{k}", tag=f"ss{k}")
                nc.vector.memset(ss[0:1, 0:1], 0.0)
                nc.vector.tensor_copy(out=ss[0:1, 1:P], in_=cslot(k)[0:1, 0:P - 1])
                d1 = mp.tile([1, P], F32, name=f"d1_{k}", tag=f"d1_{k}")
                nc.vector.tensor_tensor(out=d1[:], in0=ss[:], in1=hs_f[:], op=OP.mult)
                nc.vector.tensor_tensor(out=d1[:], in0=d1[:], in1=rr[:], op=OP.mult)
                nc.vector.tensor_tensor_scan(
                    out=carryT_v[:, k], data0=d0f[:], data1=d1[:],
                    initial=0.0, op0=OP.mult, op1=OP.add)

            def cross_bwd(k):
                ss = mp.tile([1, P], F32, name=f"ssb{k}", tag=f"ssb{k}")
                nc.vector.memset(ss[0:1, P - 1:P], 0.0)
                nc.vector.tensor_copy(out=ss[0:1, 0:P - 1], in_=cslot(k)[0:1, 1:P])
                d1 = mp.tile([1, P], F32, name=f"d1b_{k}", tag=f"d1b_{k}")
                nc.vector.tensor_tensor(out=d1[:], in0=ss[:], in1=hs_b[:], op=OP.mult)
                nc.vector.tensor_tensor(out=d1[:], in0=d1[:], in1=rrb[:], op=OP.mult)
                rv = lambda ap: ap[0:1, P - 1::-1]
                nc.vector.tensor_tensor_scan(
                    out=rv(carryT_v[:, k]), data0=rv(d0b[:]),
                    data1=rv(d1[:]), initial=0.0, op0=OP.mult, op1=OP.add)

            cross_fwd(0)
            cross_bwd(1)

            carry = mp.tile([P, NSC], F32, name="carry", tag="carry")
            nc.vector.memset(carryT_v[:, 2], 0.0)
            nc.vector.memset(carryT_v[:, 3], 0.0)
            nc.sync.dma_start(carry[:, :], carryT[0:1, :])

            start = t256("start")
            ffscan(start, pv_start, carry[:, 0:1])
            end = t256("end")
            ffscan(end, pv_end, carry[:, 1:2], backward=True)

            # dependent scans: dur_prev, dur_next
            start_sh = t256("start_sh")
            nc.vector.memset(start_sh[:], 0.0)
            nc.vector.tensor_copy(out=start_sh[:, 1:CL], in_=start[:, 0:CL - 1])
            nc.sync.dma_start(start_sh[1:P, 0:1], start[0:P - 1, CL - 1:CL])
            pv_dp = t256("pv_dp")
            nc.vector.tensor_tensor(out=pv_dp[:], in0=pos[:], in1=start_sh[:],
                                    op=OP.subtract)
            nc.vector.tensor_tensor(out=pv_dp[:], in0=pv_dp[:], in1=m_s[:], op=OP.mult)
            s_dp = t256("s_dp")
            ffscan(s_dp, pv_dp, 0.0)

            end_sh = t256("end_sh")
            nc.vector.memset(end_sh[:], 0.0)
            nc.vector.tensor_copy(out=end_sh[:, 0:CL - 1], in_=end[:, 1:CL])
            nc.sync.dma_start(end_sh[0:P - 1, CL - 1:CL], end[1:P, 0:1])
            pv_dn = t256("pv_dn")
            nc.vector.scalar_tensor_tensor(out=pv_dn[:], in0=pos[:], scalar=1.0,
                                           in1=end_sh[:], op0=OP.add, op1=OP.subtract)
            neg_me = t256("neg_me")
            nc.vector.tensor_scalar(out=neg_me[:], in0=m_e[:], scalar1=-1.0,
                                    scalar2=None, op0=OP.mult)
            nc.vector.tensor_tensor(out=pv_dn[:], in0=pv_dn[:], in1=neg_me[:],
                                    op=OP.mult)
            s_dn = t256("s_dn")
            ffscan(s_dn, pv_dn, 0.0, backward=True)

            coll2 = mp.tile([P, 2], F32, name="coll2", tag="coll2")
            nc.vector.tensor_copy(out=coll2[:, 0:1], in_=s_dp[:, CL - 1:CL])
            nc.vector.tensor_copy(out=coll2[:, 1:2], in_=s_dn[:, 0:1])
            crossT2 = mp.tile([1, 2 * P], F32, name="crossT2", tag="crossT2")
            nc.sync.dma_start(crossT2[0:1, :], coll2[:, :])
            crossT2_v = crossT2[0:1, :].rearrange("a (p k) -> a k p", k=2)
            carryT2 = mp.tile([1, 2 * P], F32, name="carryT2", tag="carryT2")
            carryT2_v = carryT2[0:1, :].rearrange("a (p k) -> a k p", k=2)

            ss = mp.tile([1, P], F32, name="ss_dp", tag="ss_dp")
            nc.vector.memset(ss[0:1, 0:1], 0.0)
            nc.vector.tensor_copy(out=ss[0:1, 1:P], in_=crossT2_v[:, 0][0:1, 0:P - 1])
            d1 = mp.tile([1, P], F32, name="d1_dp", tag="d1_dp")
            nc.vector.tensor_tensor(out=d1[:], in0=ss[:], in1=hs_f[:], op=OP.mult)
            nc.vector.tensor_tensor(out=d1[:], in0=d1[:], in1=rr[:], op=OP.mult)
            nc.vector.tensor_tensor_scan(out=carryT2_v[:, 0], data0=d0f[:],
                                         data1=d1[:], initial=0.0,
                                         op0=OP.mult, op1=OP.add)

            ss2 = mp.tile([1, P], F32, name="ss_dn", tag="ss_dn")
            nc.vector.memset(ss2[0:1, P - 1:P], 0.0)
            nc.vector.tensor_copy(out=ss2[0:1, 0:P - 1], in_=crossT2_v[:, 1][0:1, 1:P])
            d12 = mp.tile([1, P], F32, name="d1_dn", tag="d1_dn")
            nc.vector.tensor_tensor(out=d12[:], in0=ss2[:], in1=hs_b[:], op=OP.mult)
            nc.vector.tensor_tensor(out=d12[:], in0=d12[:], in1=rrb[:], op=OP.mult)
            rv = lambda ap: ap[0:1, P - 1::-1]
            nc.vector.tensor_tensor_scan(out=rv(carryT2_v[:, 1]), data0=rv(d0b[:]),
                                         data1=rv(d12[:]), initial=0.0,
                                         op0=OP.mult, op1=OP.add)

            carry2 = mp.tile([P, 2], F32, name="carry2", tag="carry2")
            nc.sync.dma_start(carry2[:, :], carryT2[0:1, :])

            dur_prev = t256("dur_prev")
            ffscan(dur_prev, pv_dp, carry2[:, 0:1])
            dur_next = t256("dur_next")
            ffscan(dur_next, pv_dn, carry2[:, 1:2], backward=True)

            # ---------- weights ----------
            dur = t256("dur")
            nc.vector.tensor_tensor(out=dur[:], in0=end[:], in1=start[:],
                                    op=OP.subtract)

            def side(dmin_a, dmin_b, bnd, bnd_cmp_imm, bnd_op, pos_side):
                tg = pos_side
                mn = t256("mn_" + tg)
                nc.vector.tensor_tensor(out=mn[:], in0=dmin_a[:], in1=dmin_b[:],
                                        op=OP.min)
                rad = t256("rad_" + tg)
                nc.vector.tensor_scalar(out=rad[:], in0=mn[:], scalar1=0.3,
                                        scalar2=None, op0=OP.mult)
                rr_ = t256("r_" + tg)
                nc.vector.tensor_scalar(out=rr_[:], in0=rad[:], scalar1=MAGIC,
                                        scalar2=MAGIC, op0=OP.add, op1=OP.subtract)
                nc.vector.tensor_scalar(out=rr_[:], in0=rr_[:], scalar1=1.0,
                                        scalar2=None, op0=OP.max)
                vbnd = t256("vbnd_" + tg)
                nc.vector.tensor_scalar(out=vbnd[:], in0=bnd[:], scalar1=bnd_cmp_imm,
                                        scalar2=None, op0=bnd_op)
                vrad = t256("vrad_" + tg)
                nc.vector.tensor_scalar(out=vrad[:], in0=rad[:], scalar1=0.5,
                                        scalar2=None, op0=OP.is_ge)
                valid = t256("valid_" + tg)
                nc.vector.tensor_tensor(out=valid[:], in0=vbnd[:], in1=vrad[:],
                                        op=OP.mult)
                num = t256("num_" + tg)
                if pos_side == "n":
                    ls = t256("ls_n")
                    nc.vector.tensor_tensor(out=ls[:], in0=end[:], in1=rr_[:],
                                            op=OP.subtract)
                    nc.vector.tensor_scalar(out=ls[:], in0=ls[:], scalar1=0.0,
                                            scalar2=None, op0=OP.max)
                    nc.vector.scalar_tensor_tensor(out=num[:], in0=pos[:], scalar=1.0,
                                                   in1=ls[:], op0=OP.add,
                                                   op1=OP.subtract)
                else:
                    re = t256("re_p")
                    nc.vector.tensor_tensor(out=re[:], in0=start[:], in1=rr_[:],
                                            op=OP.add)
                    nc.vector.tensor_scalar(out=re[:], in0=re[:], scalar1=float(T),
                                            scalar2=None, op0=OP.min)
                    nc.vector.tensor_tensor(out=num[:], in0=re[:], in1=pos[:],
                                            op=OP.subtract)
                inm = t256("inm_" + tg)
                nc.vector.tensor_scalar(out=inm[:], in0=num[:], scalar1=1.0,
                                        scalar2=None, op0=OP.is_ge)
                nc.vector.tensor_tensor(out=inm[:], in0=inm[:], in1=valid[:],
                                        op=OP.mult)
                nt = t256("nt_" + tg)
                nc.vector.tensor_tensor(out=nt[:], in0=num[:], in1=rr_[:], op=OP.min)
                nc.vector.tensor_tensor(out=nt[:], in0=nt[:], in1=inm[:], op=OP.mult)
                rcp = t256("rcp_" + tg)
                nc.vector.reciprocal(out=rcp[:], in_=rr_[:])
                wd = t256("wd_" + tg)
                nc.vector.tensor_scalar(out=wd[:], in0=num[:], scalar1=0.5,
                                        scalar2=None, op0=OP.mult)
                nc.vector.tensor_tensor(out=wd[:], in0=wd[:], in1=rcp[:], op=OP.mult)
                w_ = t256("w_" + tg)
                nc.vector.scalar_tensor_tensor(out=w_[:], in0=wd[:], scalar=0.5,
                                               in1=inm[:], op0=OP.min, op1=OP.mult)
                return w_, nt, rr_

            w_n, nt_n, r_n = side(dur, dur_next, end, float(T), OP.is_lt, "n")
            w_p, nt_p, r_p = side(dur_prev, dur, start, 0.0, OP.is_gt, "p")

            w = t256("w")
            nc.vector.tensor_tensor(out=w[:], in0=w_p[:], in1=w_n[:], op=OP.max)

            a_ = t256("a_")
            nc.vector.tensor_tensor(out=a_[:], in0=nt_n[:], in1=r_p[:], op=OP.mult)
            b_ = t256("b_")
            nc.vector.tensor_tensor(out=b_[:], in0=nt_p[:], in1=r_n[:], op=OP.mult)
            seln = t256("seln", I32)
            nc.vector.tensor_tensor(out=seln[:], in0=a_[:], in1=b_[:], op=OP.is_gt)
            selp = t256("selp", I32)
            nc.vector.tensor_scalar(out=selp[:], in0=nt_p[:], scalar1=0.0,
                                    scalar2=None, op0=OP.is_gt)

            # neighbor-representative position g (prev: start-1, next: end)
            g_t = t256("g_t")
            nc.vector.tensor_copy(out=g_t[:], in_=pos[:])
            sm1 = t256("sm1")
            nc.vector.tensor_scalar(out=sm1[:], in0=start[:], scalar1=-1.0,
                                    scalar2=None, op0=OP.add)
            nc.vector.copy_predicated(out=g_t[:], mask=selp[:], data=sm1[:])
            nc.vector.copy_predicated(out=g_t[:], mask=seln[:], data=end[:])

            # g_rel = g - 512*(chunk//2), row-local group offset
            cvals_i = mp.tile([P, 1], I32, name="cvals_i", tag="cvals_i")
            nc.vector.tensor_scalar(out=cvals_i[:], in0=ip_i[:], scalar1=30,
                                    scalar2=None, op0=OP.bitwise_and)
            cvals = mp.tile([P, 1], F32, name="cvals", tag="cvals")
            nc.vector.tensor_copy(out=cvals[:], in_=cvals_i[:])
            nc.vector.tensor_scalar(out=cvals[:], in0=cvals[:], scalar1=-256.0,
                                    scalar2=None, op0=OP.mult)
            g_rel = t256("g_rel", F16)
            nc.vector.tensor_tensor(out=g_rel[:], in0=g_t[:],
                                    in1=cvals[:].to_broadcast([P, CL]), op=OP.add)
            w16 = t256("w16", F16)
            nc.vector.tensor_copy(out=w16[:], in_=w[:])
            # bounce g_rel and w rows through DRAM; stream small partition-0
            # staging slices back per group (broadcast matmul rhs needs base 0)
            gw_d = nc.dram_tensor("gw_bounce", [2, P * CL], F16)
            nc.sync.dma_start(gw_d[0:1, :], g_rel[:, :])
            nc.sync.dma_start(gw_d[1:2, :], w16[:, :])

            for ch in range(NCH):
                nc.gpsimd.dma_gather(
                    out_ap=wins[:, ch * GCH * D:(ch + 1) * GCH * D].rearrange(
                        "p (g d) -> p g d", d=D),
                    in_ap=tbl_d[:],
                    idxs_ap=widx[:, ch * (GCH * WIN // 16):
                                 (ch + 1) * (GCH * WIN // 16)],
                    num_idxs=GCH * WIN,
                    num_idxs_reg=GCH * WIN,
                    elem_size=D, single_packet=False)

            # ---------- static matrices ----------
            ident = mp.tile([P, P], F32, name="ident", tag="ident")
            make_identity(nc, ident[:])
            # lmat = I - superdiag  (delta = lmat^T @ wins)
            idsh = mp.tile([P, P], F32, name="idsh", tag="idsh")
            nc.vector.memset(idsh[:, 0:1], 0.0)
            nc.vector.tensor_copy(out=idsh[:, 1:P], in_=ident[:, 0:P - 1])
            lmat = mp.tile([P, P], F32, name="lmat", tag="lmat")
            nc.vector.tensor_tensor(out=lmat[:], in0=ident[:], in1=idsh[:],
                                    op=OP.subtract)
            ones1 = mp.tile([1, P], F16, name="ones1", tag="ones1")
            nc.vector.memset(ones1[:], 1.0)

            # ---------- delta via PE: 2 matmuls of N=512 per chunk, f32 ------
            for ch in range(NCH):
                for hh in range(2):
                    dps = pgp.tile([P, GP], F32, name=f"dps{ch}{hh}", tag="gbc")
                    nc.tensor.matmul(
                        out=dps[:],
                        lhsT=lmat[:],
                        rhs=wins[:, (2 * ch + hh) * GP:(2 * ch + hh + 1) * GP],
                        start=True, stop=True)
                    nc.scalar.copy(out=deltas[ch][:, hh * GP:(hh + 1) * GP],
                                   in_=dps[:])

            # ---------- per-group expansion ----------
            out_v = out_d[:].rearrange("r (gr s q) d -> r gr q s d", s=4, q=P)
            stg2 = None
            for ga in range(NGA):
                r, gr = ga // NGR, ga % NGR
                # stage g and w rows for a PAIR of groups into partition 0
                if ga % 2 == 0:
                    stg2 = sgp.tile([1, 4 * GP], F16, name=f"st{ga}", tag="stg")
                    nc.sync.dma_start(stg2[0:1, 0:2 * GP],
                                      gw_d[0:1, ga * GP:(ga + 2) * GP])
                    nc.sync.dma_start(stg2[0:1, 2 * GP:4 * GP],
                                      gw_d[1:2, ga * GP:(ga + 2) * GP])
                h2 = ga % 2
                # broadcast g_rel and w rows into PSUM [128, 512] each
                gbc = pgp.tile([P, GP], F32, name=f"gbc{ga}", tag="gbc")
                nc.tensor.matmul(out=gbc[:],
                                 lhsT=ones1[:],
                                 rhs=stg2[0:1, h2 * GP:(h2 + 1) * GP],
                                 start=True, stop=True)
                wbc = pp.tile([P, GP], F32, name=f"wbc{ga}", tag="wbc")
                nc.tensor.matmul(out=wbc[:],
                                 lhsT=ones1[:],
                                 rhs=stg2[0:1, (2 + h2) * GP:(3 + h2) * GP],
                                 start=True, stop=True)
                # step matrices on ACT (saturated sigmoid)
                s_t = sgp.tile([P, GP], F16, name=f"s{ga}", tag="s_t")
                nc.scalar.activation(out=s_t[:], in_=iq[:],
                                     func=mybir.ActivationFunctionType.Sigmoid,
                                     bias=bneg[:, ga:ga + 1], scale=SC)
                sg_t = sgp.tile([P, GP], F16, name=f"sg{ga}", tag="sg_t")
                nc.scalar.activation(out=sg_t[:], in_=gbc[:],
                                     func=mybir.ActivationFunctionType.Sigmoid,
                                     bias=bneg[:, ga:ga + 1], scale=SC)
                # A = S + w*(Sg - S) on DVE (fp16)
                a_t = sgp.tile([P, GP], F16, name=f"a{ga}", tag="a_t")
                nc.vector.tensor_tensor(out=a_t[:], in0=sg_t[:], in1=s_t[:],
                                        op=OP.subtract)
                nc.vector.tensor_tensor(out=a_t[:], in0=a_t[:], in1=wbc[:],
                                        op=OP.mult)
                nc.vector.tensor_tensor(out=a_t[:], in0=a_t[:], in1=s_t[:],
                                        op=OP.add)
                # expansion matmuls: 4 M-tiles of 128 positions -> final rows
                o_ps = pp.tile([P, 4 * D], F32, name=f"o{ga}", tag="o_ps")
                for m in range(4):
                    nc.tensor.matmul(out=o_ps[:, m * D:(m + 1) * D],
                                     lhsT=a_t[:, m * P:(m + 1) * P],
                                     rhs=deltas[ga // GCH][:].rearrange(
                                         "p (g d) -> p g d", d=D)[:, ga % GCH, :],
                                     start=True, stop=True)
                ob = obp.tile([P, 4 * D], F32, name=f"ob{ga}", tag="ob")
                nc.scalar.copy(out=ob[:], in_=o_ps[:])
                nc.sync.dma_start(out_v[r, gr],
                                  ob[:].rearrange("p (s d) -> p s d", d=D))
                if dbg and ga == 0:
                    for nm, tl in [("s0", s_t), ("sg0", sg_t)]:
                        dd = nc.dram_tensor(f"dbg_{nm}", list(tl.shape), tl.dtype,
                                            kind="ExternalOutput")
                        nc.sync.dma_start(dd[:], tl[:])

            if dbg:
                for nm, tl in [("w", w), ("g_rel", g_rel),
                               ("delta", delta), ("wins", wins),
                               ("start", start), ("end", end)]:
                    dd = nc.dram_tensor(f"dbg_{nm}", list(tl.shape), tl.dtype,
                                        kind="ExternalOutput")
                    nc.sync.dma_start(dd[:], tl[:])

    nc.finalize()
    return nc


_NC_CACHE = None
LAST_RESULTS = None


def _host_prep(ids_core):
    """Window tables for one core: widx int16 wrapped + bneg f32."""
    sid_w = np.zeros((R, NGR, WIN), np.int32)
    b_rel = np.full((R, NGR, WIN), BIGNEG, np.float32)
    for r in range(R):
        row = ids_core[r]
        bnd = np.nonzero(np.diff(row) != 0)[0] + 1
        starts = np.concatenate([[0], bnd]).astype(np.int64)
        sids = row[starts]
        ns = len(sids)
        seg_of = np.searchsorted(starts, np.arange(0, T, GP), side="right") - 1
        for g in range(NGR):
            s0 = seg_of[g]
            base = max(s0 - 1, 0)
            hi = min(base + WIN, ns)
            n = hi - base
            sid_w[r, g, :n] = sids[base:hi]
            sid_w[r, g, n:] = sids[hi - 1]
            rel = (starts[base:hi] - g * GP).astype(np.float32)
            b_rel[r, g, :n] = rel
            b_rel[r, g, 0] = BIGNEG
            b_rel[r, g, n:] = 4096.0
    # widx: gathered row i = ga*WIN + j -> (partition j, slot ga)
    flat = sid_w.reshape(NGA, WIN).astype(np.int16)    # [ga, j]
    idx_lin = flat.reshape(-1)                          # i = ga*128 + j
    wrapped = idx_lin.reshape(NGA * WIN // 16, 16)
    w16 = np.ascontiguousarray(wrapped.T)               # [16, NGA*WIN/16]
    widx = np.ascontiguousarray(np.tile(w16, (8, 1)))
    # bneg[j, ga] = SC*(0.5 - b_rel)
    bneg = np.ascontiguousarray(
        (SC * (0.5 - b_rel.reshape(NGA, WIN))).T.astype(np.float32))
    return widx, bneg


def kernel(ids, table):
    global _NC_CACHE, LAST_RESULTS
    ids = np.asarray(ids)
    table = np.ascontiguousarray(np.asarray(table, dtype=np.float32))
    assert ids.shape == (B, T) and table.shape == (V, D)
    ids32 = np.ascontiguousarray(ids.astype(np.int32))

    if _NC_CACHE is None:
        _NC_CACHE = build_nc()
    nc = _NC_CACHE

    in_maps = []
    for c in range(NCORES):
        ic = ids32[c * R:(c + 1) * R]
        widx, bneg = _host_prep(ic)
        in_maps.append({"ids": ic, "widx": widx, "bneg": bneg, "table": table})

    import os
    kw = {}
    td = os.environ.get("KERNEL_TRACE_DIR")
    if td:
        os.makedirs(td, exist_ok=True)
        kw["tmpdir"] = td
    res = run_bass_kernel_spmd(nc, in_maps, list(range(NCORES)), **kw)
    LAST_RESULTS = res
    out = np.concatenate([res.results[c]["out"] for c in range(NCORES)], axis=0)
    return out.astype(np.float32)


# revision 24
# speedup vs baseline: 1.0480x; 1.0480x over previous
"""BlurredPhonemeEmbedding TRN2 kernel v2 — windowed step-matmul expansion.

Full inputs: ids (32, 8192) int64, table (2820, 64) f32.
Output: (32, 8192, 64) f32 = (1-w)*tbl[ids] + w*tbl[neighbor].

Data-parallel over batch: 8 cores x 4 rows; table replicated.

Device algorithm per core (R=4 rows, T=8192):
 - scan layout [128, 256]: partition p = r*32 + c, free j; t_row = c*256 + j.
 - segment quantities (start/end/dur/dur_prev/dur_next) via masked
   fill-forward scans with cross-chunk carries (as v1), minus the id scans.
 - blend weight w and neighbor-representative position g per position.
 - HOST preps per-(row,group) segment windows (pure index data):
   group = 512 positions, window = 128 segment slots [s0-1, s0+126];
   widx: wrapped int16 ids for ONE dma_gather of 64*128 window rows;
   bneg[j, ga] = 32*(0.5 - (sstart[s] - 512*g)) for ACT sigmoid bias.
 - emb[t] = sum_j step(q >= b_j) * Delta_j  (telescoping over window rows),
   nemb[t] same with g_rel instead of q. Steps built on ACT via saturated
   sigmoid (scale 32 -> exact 0/1 in fp16); expansion via PE fp16 matmuls
   accumulating f32 in PSUM. Blend on DVE in f32; per-group 128KB stores.
"""
import numpy as np

import concourse.bass as bass
import concourse.tile as tile
from concourse import bacc, mybir
from concourse.bass_utils import run_bass_kernel_spmd
from concourse.masks import make_identity

F32 = mybir.dt.float32
F16 = mybir.dt.float16
I32 = mybir.dt.int32
I16 = mybir.dt.int16
OP = mybir.AluOpType

B, T, V, D = 32, 8192, 2820, 64
NCORES = 8
R = B // NCORES            # rows per core = 4
P = 128                    # partitions
CPR = P // R               # chunks per row = 32
CL = T // CPR              # chunk length = 256
GP = 512                   # positions per group
NGR = T // GP              # groups per row = 16
NGA = R * NGR              # groups per core = 64
WIN = 128                  # window slots per group
MAGIC = float(2 ** 23)
BIGNEG = -2048.0
SC = 32.0                  # sigmoid step scale


def build_nc(dbg=False):
    nc = bacc.Bacc("TRN2", target_bir_lowering=False, debug=False,
                   dynamic_dma_scratch_size=16384)
    ids_d = nc.dram_tensor("ids", [R, T], I32, kind="ExternalInput")
    widx_d = nc.dram_tensor("widx", [P, NGA * WIN // 16], I16, kind="ExternalInput")
    bneg_d = nc.dram_tensor("bneg", [P, NGA], F32, kind="ExternalInput")
    tbl_d = nc.dram_tensor("table", [V, D], F32, kind="ExternalInput")
    out_d = nc.dram_tensor("out", [R, T, D], F32, kind="ExternalOutput")

    with tile.TileContext(nc) as tc:
        with tc.tile_pool(name="main", bufs=1) as mp, \
             tc.tile_pool(name="sg", bufs=6) as sgp, \
             tc.tile_pool(name="ob", bufs=6) as obp, \
             tc.tile_pool(name="ps", bufs=3, space="PSUM") as pp, \
             tc.tile_pool(name="psg", bufs=2, space="PSUM") as pgp:

            def t256(name, dt=F32):
                return mp.tile([P, CL], dt, name=name, tag=name)

            # ---------- loads ----------
            ids_i = t256("ids_i", I32)
            nc.sync.dma_start(ids_i[:], ids_d[:].rearrange("r (c j) -> (r c) j", j=CL))
            widx = mp.tile([P, NGA * WIN // 16], I16, name="widx_t", tag="widx_t")
            nc.sync.dma_start(widx[:], widx_d[:])
            bneg = mp.tile([P, NGA], F32, name="bneg_t", tag="bneg_t")
            nc.sync.dma_start(bneg[:], bneg_d[:])

            # static iotas (gpsimd first, before the big gather)
            pos_i = t256("pos_i", I32)
            nc.gpsimd.iota(pos_i[:], pattern=[[1, CL]], base=0, channel_multiplier=CL)
            iq_i = mp.tile([P, GP], I32, name="iq_i", tag="iq_i")
            nc.gpsimd.iota(iq_i[:], pattern=[[1, GP]], base=0, channel_multiplier=0)
            iq = mp.tile([P, GP], F32, name="iq", tag="iq")
            nc.vector.tensor_copy(out=iq[:], in_=iq_i[:])
            ip_i = mp.tile([P, 1], I32, name="ip_i", tag="ip_i")
            nc.gpsimd.iota(ip_i[:], pattern=[[0, 1]], base=0, channel_multiplier=1)
            ip = mp.tile([P, 1], F32, name="ip", tag="ip")
            nc.vector.tensor_copy(out=ip[:], in_=ip_i[:])

            # Window-row gather happens in 4 chunks of 2048 rows, issued AFTER
            # the scan pipeline (see below): the gpsimd engine is in-order, so
            # a tiny gpsimd op depending on the last scan output fences the
            # gathers behind the scans — Q7 SWDGE descriptor generation would
            # otherwise slow the DVE scans 2-4x via SBUF port contention.
            wins = mp.tile([P, NGA * D], F32, name="wins", tag="wins")
            NCH = 4
            GCH = NGA // NCH           # 16 groups per chunk
            # per-chunk delta tiles so early groups only wait on their chunk
            deltas = [mp.tile([P, GCH * D], F16, name=f"delta{ch}",
                              tag=f"delta{ch}") for ch in range(NCH)]

            # ---------- pos / masks / scans (v1 machinery minus id scans) ----
            nc.vector.tensor_scalar(out=pos_i[:], in0=pos_i[:], scalar1=T - 1,
                                    scalar2=None, op0=OP.bitwise_and)
            pos = t256("pos")
            nc.vector.tensor_copy(out=pos[:], in_=pos_i[:])

            ids_f = t256("ids_f")
            nc.vector.tensor_copy(out=ids_f[:], in_=ids_i[:])

            ids_prev = t256("ids_prev")
            nc.vector.memset(ids_prev[:], 0.0)
            nc.vector.tensor_copy(out=ids_prev[:, 1:CL], in_=ids_f[:, 0:CL - 1])
            nc.sync.dma_start(ids_prev[1:P, 0:1], ids_f[0:P - 1, CL - 1:CL])

            ids_next = t256("ids_next")
            nc.vector.memset(ids_next[:], 0.0)
            nc.vector.tensor_copy(out=ids_next[:, 0:CL - 1], in_=ids_f[:, 1:CL])
            nc.sync.dma_start(ids_next[0:P - 1, CL - 1:CL], ids_f[1:P, 0:1])

            m_s = t256("m_s")
            nc.vector.tensor_tensor(out=m_s[:], in0=ids_f[:], in1=ids_prev[:],
                                    op=OP.not_equal)
            edge_s = t256("edge_s")
            nc.vector.tensor_scalar(out=edge_s[:], in0=pos[:], scalar1=0.0,
                                    scalar2=None, op0=OP.is_equal)
            nc.vector.tensor_tensor(out=m_s[:], in0=m_s[:], in1=edge_s[:], op=OP.max)
            m_e = t256("m_e")
            nc.vector.tensor_tensor(out=m_e[:], in0=ids_f[:], in1=ids_next[:],
                                    op=OP.not_equal)
            edge_e = t256("edge_e")
            nc.vector.tensor_scalar(out=edge_e[:], in0=pos[:], scalar1=float(T - 1),
                                    scalar2=None, op0=OP.is_equal)
            nc.vector.tensor_tensor(out=m_e[:], in0=m_e[:], in1=edge_e[:], op=OP.max)

            om_s = t256("om_s")
            nc.vector.tensor_scalar(out=om_s[:], in0=m_s[:], scalar1=-1.0, scalar2=1.0,
                                    op0=OP.mult, op1=OP.add)
            om_e = t256("om_e")
            nc.vector.tensor_scalar(out=om_e[:], in0=m_e[:], scalar1=-1.0, scalar2=1.0,
                                    op0=OP.mult, op1=OP.add)

            def rev(ap):
                return ap[:, CL - 1::-1]

            def ffscan(out_t, d1, initial, backward=False):
                om = om_e if backward else om_s
                if backward:
                    nc.vector.tensor_tensor_scan(
                        out=rev(out_t[:]), data0=rev(om[:]), data1=rev(d1[:]),
                        initial=initial, op0=OP.mult, op1=OP.add)
                else:
                    nc.vector.tensor_tensor_scan(
                        out=out_t[:], data0=om[:], data1=d1[:],
                        initial=initial, op0=OP.mult, op1=OP.add)

            pv_start = t256("pv_start")
            nc.vector.tensor_tensor(out=pv_start[:], in0=pos[:], in1=m_s[:], op=OP.mult)
            pv_end = t256("pv_end")
            nc.vector.scalar_tensor_tensor(out=pv_end[:], in0=pos[:], scalar=1.0,
                                           in1=m_e[:], op0=OP.add, op1=OP.mult)

            s_start = t256("s_start")
            ffscan(s_start, pv_start, 0.0)
            s_end = t256("s_end")
            ffscan(s_end, pv_end, 0.0, backward=True)

            # cross-chunk carries: [128, 4] -> [1, 512] transposed view
            NSC = 4
            coll = mp.tile([P, NSC], F32, name="coll", tag="coll")
            nc.vector.tensor_copy(out=coll[:, 0:1], in_=s_start[:, CL - 1:CL])
            nc.vector.tensor_copy(out=coll[:, 1:2], in_=s_end[:, 0:1])
            nc.vector.tensor_reduce(out=coll[:, 2:3], in_=m_s[:],
                                    axis=mybir.AxisListType.X, op=OP.max)
            nc.vector.tensor_reduce(out=coll[:, 3:4], in_=m_e[:],
                                    axis=mybir.AxisListType.X, op=OP.max)

            crossT = mp.tile([1, NSC * P], F32, name="crossT", tag="crossT")
            nc.sync.dma_start(crossT[0:1, :], coll[:, :])
            crossT_v = crossT[0:1, :].rearrange("a (p k) -> a k p", k=NSC)

            def cslot(k):
                return crossT_v[:, k]

            rr = mp.tile([1, P], F32, name="rr", tag="rr")
            nc.vector.memset(rr[:], 1.0)
            rrb = mp.tile([1, P], F32, name="rrb", tag="rrb")
            nc.vector.memset(rrb[:], 1.0)
            for r in range(R):
                nc.vector.memset(rr[0:1, r * CPR:r * CPR + 1], 0.0)
                nc.vector.memset(rrb[0:1, (r + 1) * CPR - 1:(r + 1) * CPR], 0.0)

            hs_f = mp.tile([1, P], F32, name="hs_f", tag="hs_f")
            nc.vector.memset(hs_f[0:1, 0:1], 0.0)
            nc.vector.tensor_copy(out=hs_f[0:1, 1:P], in_=cslot(2)[0:1, 0:P - 1])
            d0f = mp.tile([1, P], F32, name="d0f", tag="d0f")
            nc.vector.tensor_scalar(out=d0f[:], in0=hs_f[:], scalar1=-1.0, scalar2=1.0,
                                    op0=OP.mult, op1=OP.add)
            nc.vector.tensor_tensor(out=d0f[:], in0=d0f[:], in1=rr[:], op=OP.mult)
            hs_b = mp.tile([1, P], F32, name="hs_b", tag="hs_b")
            nc.vector.memset(hs_b[0:1, P - 1:P], 0.0)
            nc.vector.tensor_copy(out=hs_b[0:1, 0:P - 1], in_=cslot(3)[0:1, 1:P])
            d0b = mp.tile([1, P], F32, name="d0b", tag="d0b")
            nc.vector.tensor_scalar(out=d0b[:], in0=hs_b[:], scalar1=-1.0, scalar2=1.0,
                                    op0=OP.mult, op1=OP.add)
            nc.vector.tensor_tensor(out=d0b[:], in0=d0b[:], in1=rrb[:], op=OP.mult)

            carryT = mp.tile([1, NSC * P], F32, name="carryT", tag="carryT")
            carryT_v = carryT[0:1, :].rearrange("a (p k) -> a k p", k=NSC)

            def cross_fwd(k):
                ss = mp.tile([1, P], F32, name=f"ss{k}", tag=f"ss{k}")
                nc.vector.memset(ss[0:1, 0:1], 0.0)
                nc.vector.tensor_copy(out=ss[0:1, 1:P], in_=cslot(k)[0:1, 0:P - 1])
                d1 = mp.tile([1, P], F32, name=f"d1_{k}", tag=f"d1_{k}")
                nc.vector.tensor_tensor(out=d1[:], in0=ss[:], in1=hs_f[:], op=OP.mult)
                nc.vector.tensor_tensor(out=d1[:], in0=d1[:], in1=rr[:], op=OP.mult)
                nc.vector.tensor_tensor_scan(
                    out=carryT_v[:, k], data0=d0f[:], data1=d1[:],
                    initial=0.0, op0=OP.mult, op1=OP.add)

            def cross_bwd(k):
                ss = mp.tile([1, P], F32, name=f"ssb{k}", tag=f"ssb{k}")
                nc.vector.memset(ss[0:1, P - 1:P], 0.0)
                nc.vector.tensor_copy(out=ss[0:1, 0:P - 1], in_=cslot(k)[0:1, 1:P])
                d1 = mp.tile([1, P], F32, name=f"d1b_{k}", tag=f"d1b_{k}")
                nc.vector.tensor_tensor(out=d1[:], in0=ss[:], in1=hs_b[:], op=OP.mult)
                nc.vector.tensor_tensor(out=d1[:], in0=d1[:], in1=rrb[:], op=OP.mult)
                rv = lambda ap: ap[0:1, P - 1::-1]
                nc.vector.tensor_tensor_scan(
                    out=rv(carryT_v[:, k]), data0=rv(d0b[:]),
                    data1=rv(d1[:]), initial=0.0, op0=OP.mult, op1=OP.add)

            cross_fwd(0)
            cross_bwd(1)

            carry = mp.tile([P, NSC], F32, name="carry", tag="carry")
            nc.vector.memset(carryT_v[:, 2], 0.0)
            nc.vector.memset(carryT_v[:, 3], 0.0)
            nc.sync.dma_start(carry[:, :], carryT[0:1, :])

            start = t256("start")
            ffscan(start, pv_start, carry[:, 0:1])
            end = t256("end")
            ffscan(end, pv_end, carry[:, 1:2], backward=True)

            # dependent scans: dur_prev, dur_next
            start_sh = t256("start_sh")
            nc.vector.memset(start_sh[:], 0.0)
            nc.vector.tensor_copy(out=start_sh[:, 1:CL], in_=start[:, 0:CL - 1])
            nc.sync.dma_start(start_sh[1:P, 0:1], start[0:P - 1, CL - 1:CL])
            pv_dp = t256("pv_dp")
            nc.vector.tensor_tensor(out=pv_dp[:], in0=pos[:], in1=start_sh[:],
                                    op=OP.subtract)
            nc.vector.tensor_tensor(out=pv_dp[:], in0=pv_dp[:], in1=m_s[:], op=OP.mult)
            s_dp = t256("s_dp")
            ffscan(s_dp, pv_dp, 0.0)

            end_sh = t256("end_sh")
            nc.vector.memset(end_sh[:], 0.0)
            nc.vector.tensor_copy(out=end_sh[:, 0:CL - 1], in_=end[:, 1:CL])
            nc.sync.dma_start(end_sh[0:P - 1, CL - 1:CL], end[1:P, 0:1])
            pv_dn = t256("pv_dn")
            nc.vector.scalar_tensor_tensor(out=pv_dn[:], in0=pos[:], scalar=1.0,
                                           in1=end_sh[:], op0=OP.add, op1=OP.subtract)
            neg_me = t256("neg_me")
            nc.vector.tensor_scalar(out=neg_me[:], in0=m_e[:], scalar1=-1.0,
                                    scalar2=None, op0=OP.mult)
            nc.vector.tensor_tensor(out=pv_dn[:], in0=pv_dn[:], in1=neg_me[:],
                                    op=OP.mult)
            s_dn = t256("s_dn")
            ffscan(s_dn, pv_dn, 0.0, backward=True)

            coll2 = mp.tile([P, 2], F32, name="coll2", tag="coll2")
            nc.vector.tensor_copy(out=coll2[:, 0:1], in_=s_dp[:, CL - 1:CL])
            nc.vector.tensor_copy(out=coll2[:, 1:2], in_=s_dn[:, 0:1])
            crossT2 = mp.tile([1, 2 * P], F32, name="crossT2", tag="crossT2")
            nc.sync.dma_start(crossT2[0:1, :], coll2[:, :])
            crossT2_v = crossT2[0:1, :].rearrange("a (p k) -> a k p", k=2)
            carryT2 = mp.tile([1, 2 * P], F32, name="carryT2", tag="carryT2")
            carryT2_v = carryT2[0:1, :].rearrange("a (p k) -> a k p", k=2)

            ss = mp.tile([1, P], F32, name="ss_dp", tag="ss_dp")
            nc.vector.memset(ss[0:1, 0:1], 0.0)
            nc.vector.tensor_copy(out=ss[0:1, 1:P], in_=crossT2_v[:, 0][0:1, 0:P - 1])
            d1 = mp.tile([1, P], F32, name="d1_dp", tag="d1_dp")
            nc.vector.tensor_tensor(out=d1[:], in0=ss[:], in1=hs_f[:], op=OP.mult)
            nc.vector.tensor_tensor(out=d1[:], in0=d1[:], in1=rr[:], op=OP.mult)
            nc.vector.tensor_tensor_scan(out=carryT2_v[:, 0], data0=d0f[:],
                                         data1=d1[:], initial=0.0,
                                         op0=OP.mult, op1=OP.add)

            ss2 = mp.tile([1, P], F32, name="ss_dn", tag="ss_dn")
            nc.vector.memset(ss2[0:1, P - 1:P], 0.0)
            nc.vector.tensor_copy(out=ss2[0:1, 0:P - 1], in_=crossT2_v[:, 1][0:1, 1:P])
            d12 = mp.tile([1, P], F32, name="d1_dn", tag="d1_dn")
            nc.vector.tensor_tensor(out=d12[:], in0=ss2[:], in1=hs_b[:], op=OP.mult)
            nc.vector.tensor_tensor(out=d12[:], in0=d12[:], in1=rrb[:], op=OP.mult)
            rv = lambda ap: ap[0:1, P - 1::-1]
            nc.vector.tensor_tensor_scan(out=rv(carryT2_v[:, 1]), data0=rv(d0b[:]),
                                         data1=rv(d12[:]), initial=0.0,
                                         op0=OP.mult, op1=OP.add)

            carry2 = mp.tile([P, 2], F32, name="carry2", tag="carry2")
            nc.sync.dma_start(carry2[:, :], carryT2[0:1, :])

            dur_prev = t256("dur_prev")
            ffscan(dur_prev, pv_dp, carry2[:, 0:1])
            dur_next = t256("dur_next")
            ffscan(dur_next, pv_dn, carry2[:, 1:2], backward=True)

            # ---------- weights ----------
            dur = t256("dur")
            nc.vector.tensor_tensor(out=dur[:], in0=end[:], in1=start[:],
                                    op=OP.subtract)

            def side(dmin_a, dmin_b, bnd, bnd_cmp_imm, bnd_op, pos_side):
                tg = pos_side
                mn = t256("mn_" + tg)
                nc.vector.tensor_tensor(out=mn[:], in0=dmin_a[:], in1=dmin_b[:],
                                        op=OP.min)
                rad = t256("rad_" + tg)
                nc.vector.tensor_scalar(out=rad[:], in0=mn[:], scalar1=0.3,
                                        scalar2=None, op0=OP.mult)
                rr_ = t256("r_" + tg)
                nc.vector.tensor_scalar(out=rr_[:], in0=rad[:], scalar1=MAGIC,
                                        scalar2=MAGIC, op0=OP.add, op1=OP.subtract)
                nc.vector.tensor_scalar(out=rr_[:], in0=rr_[:], scalar1=1.0,
                                        scalar2=None, op0=OP.max)
                vbnd = t256("vbnd_" + tg)
                nc.vector.tensor_scalar(out=vbnd[:], in0=bnd[:], scalar1=bnd_cmp_imm,
                                        scalar2=None, op0=bnd_op)
                vrad = t256("vrad_" + tg)
                nc.vector.tensor_scalar(out=vrad[:], in0=rad[:], scalar1=0.5,
                                        scalar2=None, op0=OP.is_ge)
                valid = t256("valid_" + tg)
                nc.vector.tensor_tensor(out=valid[:], in0=vbnd[:], in1=vrad[:],
                                        op=OP.mult)
                num = t256("num_" + tg)
                if pos_side == "n":
                    ls = t256("ls_n")
                    nc.vector.tensor_tensor(out=ls[:], in0=end[:], in1=rr_[:],
                                            op=OP.subtract)
                    nc.vector.tensor_scalar(out=ls[:], in0=ls[:], scalar1=0.0,
                                            scalar2=None, op0=OP.max)
                    nc.vector.scalar_tensor_tensor(out=num[:], in0=pos[:], scalar=1.0,
                                                   in1=ls[:], op0=OP.add,
                                                   op1=OP.subtract)
                else:
                    re = t256("re_p")
                    nc.vector.tensor_tensor(out=re[:], in0=start[:], in1=rr_[:],
                                            op=OP.add)
                    nc.vector.tensor_scalar(out=re[:], in0=re[:], scalar1=float(T),
                                            scalar2=None, op0=OP.min)
                    nc.vector.tensor_tensor(out=num[:], in0=re[:], in1=pos[:],
                                            op=OP.subtract)
                inm = t256("inm_" + tg)
                nc.vector.tensor_scalar(out=inm[:], in0=num[:], scalar1=1.0,
                                        scalar2=None, op0=OP.is_ge)
                nc.vector.tensor_tensor(out=inm[:], in0=inm[:], in1=valid[:],
                                        op=OP.mult)
                nt = t256("nt_" + tg)
                nc.vector.tensor_tensor(out=nt[:], in0=num[:], in1=rr_[:], op=OP.min)
                nc.vector.tensor_tensor(out=nt[:], in0=nt[:], in1=inm[:], op=OP.mult)
                rcp = t256("rcp_" + tg)
                nc.vector.reciprocal(out=rcp[:], in_=rr_[:])
                wd = t256("wd_" + tg)
                nc.vector.tensor_scalar(out=wd[:], in0=num[:], scalar1=0.5,
                                        scalar2=None, op0=OP.mult)
                nc.vector.tensor_tensor(out=wd[:], in0=wd[:], in1=rcp[:], op=OP.mult)
                w_ = t256("w_" + tg)
                nc.vector.scalar_tensor_tensor(out=w_[:], in0=wd[:], scalar=0.5,
                                               in1=inm[:], op0=OP.min, op1=OP.mult)
                return w_, nt, rr_

            w_n, nt_n, r_n = side(dur, dur_next, end, float(T), OP.is_lt, "n")
            w_p, nt_p, r_p = side(dur_prev, dur, start, 0.0, OP.is_gt, "p")

            w = t256("w")
            nc.vector.tensor_tensor(out=w[:], in0=w_p[:], in1=w_n[:], op=OP.max)

            a_ = t256("a_")
            nc.vector.tensor_tensor(out=a_[:], in0=nt_n[:], in1=r_p[:], op=OP.mult)
            b_ = t256("b_")
            nc.vector.tensor_tensor(out=b_[:], in0=nt_p[:], in1=r_n[:], op=OP.mult)
            seln = t256("seln", I32)
            nc.vector.tensor_tensor(out=seln[:], in0=a_[:], in1=b_[:], op=OP.is_gt)
            selp = t256("selp", I32)
            nc.vector.tensor_scalar(out=selp[:], in0=nt_p[:], scalar1=0.0,
                                    scalar2=None, op0=OP.is_gt)

            # neighbor-representative position g (prev: start-1, next: end)
            g_t = t256("g_t")
            nc.vector.tensor_copy(out=g_t[:], in_=pos[:])
            sm1 = t256("sm1")
            nc.vector.tensor_scalar(out=sm1[:], in0=start[:], scalar1=-1.0,
                                    scalar2=None, op0=OP.add)
            nc.vector.copy_predicated(out=g_t[:], mask=selp[:], data=sm1[:])
            nc.vector.copy_predicated(out=g_t[:], mask=seln[:], data=end[:])

            # g_rel = g - 512*(chunk//2), row-local group offset
            cvals_i = mp.tile([P, 1], I32, name="cvals_i", tag="cvals_i")
            nc.vector.tensor_scalar(out=cvals_i[:], in0=ip_i[:], scalar1=30,
                                    scalar2=None, op0=OP.bitwise_and)
            cvals = mp.tile([P, 1], F32, name="cvals", tag="cvals")
            nc.vector.tensor_copy(out=cvals[:], in_=cvals_i[:])
            nc.vector.tensor_scalar(out=cvals[:], in0=cvals[:], scalar1=-256.0,
                                    scalar2=None, op0=OP.mult)
            g_rel = t256("g_rel", F16)
            nc.vector.tensor_tensor(out=g_rel[:], in0=g_t[:],
                                    in1=cvals[:].to_broadcast([P, CL]), op=OP.add)
            w16 = t256("w16", F16)
            nc.vector.tensor_copy(out=w16[:], in_=w[:])
            # bounce g_rel and w rows through DRAM; stream small partition-0
            # staging slices back per group (broadcast matmul rhs needs base 0)
            gw_d = nc.dram_tensor("gw_bounce", [2, P * CL], F16)
            nc.sync.dma_start(gw_d[0:1, :], g_rel[:, :])
            nc.sync.dma_start(gw_d[1:2, :], w16[:, :])

            for ch in range(NCH):
                nc.gpsimd.dma_gather(
                    out_ap=wins[:, ch * GCH * D:(ch + 1) * GCH * D].rearrange(
                        "p (g d) -> p g d", d=D),
                    in_ap=tbl_d[:],
                    idxs_ap=widx[:, ch * (GCH * WIN // 16):
                                 (ch + 1) * (GCH * WIN // 16)],
                    num_idxs=GCH * WIN,
                    num_idxs_reg=GCH * WIN,
                    elem_size=D, single_packet=False)

            # ---------- static matrices ----------
            ident = mp.tile([P, P], F32, name="ident", tag="ident")
            make_identity(nc, ident[:])
            # lmat = I - superdiag  (delta = lmat^T @ wins)
            idsh = mp.tile([P, P], F32, name="idsh", tag="idsh")
            nc.vector.memset(idsh[:, 0:1], 0.0)
            nc.vector.tensor_copy(out=idsh[:, 1:P], in_=ident[:, 0:P - 1])
            lmat = mp.tile([P, P], F32, name="lmat", tag="lmat")
            nc.vector.tensor_tensor(out=lmat[:], in0=ident[:], in1=idsh[:],
                                    op=OP.subtract)
            ones1 = mp.tile([1, P], F16, name="ones1", tag="ones1")
            nc.vector.memset(ones1[:], 1.0)

            # ---------- delta via PE: 2 matmuls of N=512 per chunk, f32 ------
            for ch in range(NCH):
                for hh in range(2):
                    dps = pp.tile([P, GP], F32, name=f"dps{ch}{hh}", tag="wbc")
                    nc.tensor.matmul(
                        out=dps[:],
                        lhsT=lmat[:],
                        rhs=wins[:, (2 * ch + hh) * GP:(2 * ch + hh + 1) * GP],
                        start=True, stop=True)
                    nc.scalar.copy(out=deltas[ch][:, hh * GP:(hh + 1) * GP],
                                   in_=dps[:])

            # ---------- per-group expansion ----------
            out_v = out_d[:].rearrange("r (gr s q) d -> r gr q s d", s=4, q=P)
            stg2 = None
            for ga in range(NGA):
                r, gr = ga // NGR, ga % NGR
                # stage g and w rows for a PAIR of groups into partition 0
                if ga % 2 == 0:
                    stg2 = sgp.tile([1, 4 * GP], F16, name=f"st{ga}", tag="stg")
                    nc.sync.dma_start(stg2[0:1, 0:2 * GP],
                                      gw_d[0:1, ga * GP:(ga + 2) * GP])
                    nc.sync.dma_start(stg2[0:1, 2 * GP:4 * GP],
                                      gw_d[1:2, ga * GP:(ga + 2) * GP])
                h2 = ga % 2
                # broadcast g_rel and w rows into PSUM [128, 512] each
                gbc = pgp.tile([P, GP], F32, name=f"gbc{ga}", tag="gbc")
                nc.tensor.matmul(out=gbc[:],
                                 lhsT=ones1[:],
                                 rhs=stg2[0:1, h2 * GP:(h2 + 1) * GP],
                                 start=True, stop=True)
                wbc = pp.tile([P, GP], F32, name=f"wbc{ga}", tag="wbc")
                nc.tensor.matmul(out=wbc[:],
                                 lhsT=ones1[:],
                                 rhs=stg2[0:1, (2 + h2) * GP:(3 + h2) * GP],
                                 start=True, stop=True)
                # step matrices on ACT (saturated sigmoid)
                s_t = sgp.tile([P, GP], F16, name=f"s{ga}", tag="s_t")
                nc.scalar.activation(out=s_t[:], in_=iq[:],
                                     func=mybir.ActivationFunctionType.Sigmoid,
                                     bias=bneg[:, ga:ga + 1], scale=SC)
                sg_t = sgp.tile([P, GP], F16, name=f"sg{ga}", tag="sg_t")
                nc.scalar.activation(out=sg_t[:], in_=gbc[:],
                                     func=mybir.ActivationFunctionType.Sigmoid,
                                     bias=bneg[:, ga:ga + 1], scale=SC)
                # A = S + w*(Sg - S) on DVE (fp16)
                a_t = sgp.tile([P, GP], F16, name=f"a{ga}", tag="a_t")
                nc.vector.tensor_tensor(out=a_t[:], in0=sg_t[:], in1=s_t[:],
                                        op=OP.subtract)
                nc.vector.tensor_tensor(out=a_t[:], in0=a_t[:], in1=wbc[:],
                                        op=OP.mult)
                nc.vector.tensor_tensor(out=a_t[:], in0=a_t[:], in1=s_t[:],
                                        op=OP.add)
                # expansion matmuls: 4 M-tiles of 128 positions -> final rows
                o_ps = pp.tile([P, 4 * D], F32, name=f"o{ga}", tag="o_ps")
                for m in range(4):
                    nc.tensor.matmul(out=o_ps[:, m * D:(m + 1) * D],
                                     lhsT=a_t[:, m * P:(m + 1) * P],
                                     rhs=deltas[ga // GCH][:].rearrange(
                                         "p (g d) -> p g d", d=D)[:, ga % GCH, :],
                                     start=True, stop=True)
                ob = obp.tile([P, 4 * D], F32, name=f"ob{ga}", tag="ob")
                nc.scalar.copy(out=ob[:], in_=o_ps[:])
                nc.sync.dma_start(out_v[r, gr],
                                  ob[:].rearrange("p (s d) -> p s d", d=D))
                if dbg and ga == 0:
                    for nm, tl in [("s0", s_t), ("sg0", sg_t)]:
                        dd = nc.dram_tensor(f"dbg_{nm}", list(tl.shape), tl.dtype,
                                            kind="ExternalOutput")
                        nc.sync.dma_start(dd[:], tl[:])

            if dbg:
                for nm, tl in [("w", w), ("g_rel", g_rel),
                               ("delta", delta), ("wins", wins),
                               ("start", start), ("end", end)]:
                    dd = nc.dram_tensor(f"dbg_{nm}", list(tl.shape), tl.dtype,
                                        kind="ExternalOutput")
                    nc.sync.dma_start(dd[:], tl[:])

    nc.finalize()
    return nc


_NC_CACHE = None
LAST_RESULTS = None


def _host_prep(ids_core):
    """Window tables for one core: widx int16 wrapped + bneg f32."""
    sid_w = np.zeros((R, NGR, WIN), np.int32)
    b_rel = np.full((R, NGR, WIN), BIGNEG, np.float32)
    for r in range(R):
        row = ids_core[r]
        bnd = np.nonzero(np.diff(row) != 0)[0] + 1
        starts = np.concatenate([[0], bnd]).astype(np.int64)
        sids = row[starts]
        ns = len(sids)
        seg_of = np.searchsorted(starts, np.arange(0, T, GP), side="right") - 1
        for g in range(NGR):
            s0 = seg_of[g]
            base = max(s0 - 1, 0)
            hi = min(base + WIN, ns)
            n = hi - base
            sid_w[r, g, :n] = sids[base:hi]
            sid_w[r, g, n:] = sids[hi - 1]
            rel = (starts[base:hi] - g * GP).astype(np.float32)
            b_rel[r, g, :n] = rel
            b_rel[r, g, 0] = BIGNEG
            b_rel[r, g, n:] = 4096.0
    # widx: gathered row i = ga*WIN + j -> (partition j, slot ga)
    flat = sid_w.reshape(NGA, WIN).astype(np.int16)    # [ga, j]
    idx_lin = flat.reshape(-1)                          # i = ga*128 + j
    wrapped = idx_lin.reshape(NGA * WIN // 16, 16)
    w16 = np.ascontiguousarray(wrapped.T)               # [16, NGA*WIN/16]
    widx = np.ascontiguousarray(np.tile(w16, (8, 1)))
    # bneg[j, ga] = SC*(0.5 - b_rel)
    bneg = np.ascontiguousarray(
        (SC * (0.5 - b_rel.reshape(NGA, WIN))).T.astype(np.float32))
    return widx, bneg


def kernel(ids, table):
    global _NC_CACHE, LAST_RESULTS
    ids = np.asarray(ids)
    table = np.ascontiguousarray(np.asarray(table, dtype=np.float32))
    assert ids.shape == (B, T) and table.shape == (V, D)
    ids32 = np.ascontiguousarray(ids.astype(np.int32))

    if _NC_CACHE is None:
        _NC_CACHE = build_nc()
    nc = _NC_CACHE

    in_maps = []
    for c in range(NCORES):
        ic = ids32[c * R:(c + 1) * R]
        widx, bneg = _host_prep(ic)
        in_maps.append({"ids": ic, "widx": widx, "bneg": bneg, "table": table})

    import os
    kw = {}
    td = os.environ.get("KERNEL_TRACE_DIR")
    if td:
        os.makedirs(td, exist_ok=True)
        kw["tmpdir"] = td
    res = run_bass_kernel_spmd(nc, in_maps, list(range(NCORES)), **kw)
    LAST_RESULTS = res
    out = np.concatenate([res.results[c]["out"] for c in range(NCORES)], axis=0)
    return out.astype(np.float32)
